# revision 49
# baseline (speedup 1.0000x reference)
"""Multi-head causal self-attention (GPT-style block) on 8 Trainium2 NeuronCores.

Strategy: data-parallel over batch (B=8 -> 1 batch element per core), weights
replicated. Host-side prep: x is transposed to x^T (so no PE transposes are
needed on device) and the value-projection bias is folded into the output
bias (b_proj' = b_v @ W_proj + b_proj, exact in fp32), since softmax rows
sum to 1.

Per-core dataflow (all matmul compute bf16 with fp32 PSUM accumulation):

  qT/kT [n,T] = W_attn[:, n].T-stationary matmuls over xT   (n on partitions)
  v    [T,n] = xT-stationary matmuls over W_attn v-cols     (T on partitions)
  scores^T [k,q] = kT_h.T @ qT_h  (K=64 contraction via zero-padded qT)
  P^T = exp(0.125*scores^T) via ACT, causal diag tiles masked by gpsimd mult
  out^T[d,q] & softmax denom = [v_h | ones].T @ P^T  (ones col -> denom row)
  normalize via PE-broadcast of 1/denom, DVE multiply
  y [T,H] = out^T-stationary matmuls over W_proj + bias

The QKV-projection work (q/k and v matmul chains) is software-pipelined into
the attention phase: emitted as "filler" between score-matmul groups so the
PE stays busy while ACT computes exp (PSUM ping-pong would otherwise stall
it). PSUM evacuations are spread across DVE (qk with per-partition bias add,
v copies) and ACT (exp, proj) to keep every engine below the PE's busy time.
"""

from collections import deque

import numpy as np

import concourse.bass as bass
import concourse.mybir as mybir
import concourse.tile as tile
from concourse import bacc, bass_utils
from concourse.masks import make_upper_triangular

F32 = mybir.dt.float32
BF16 = mybir.dt.bfloat16

T = 1024   # tokens per batch element
H = 768    # hidden
NH = 12    # heads
HS = 64    # head size
TT = T // 128   # token tiles (8)
FT = H // 128   # feature tiles (6)
N_CORES = 8


def build():
    nc = bacc.Bacc(None, target_bir_lowering=False)

    xT_d = nc.dram_tensor("xT", [H, T], BF16, kind="ExternalInput")
    wa_d = nc.dram_tensor("W_attn", [H, 3 * H], BF16, kind="ExternalInput")
    bqk_d = nc.dram_tensor("b_qk", [2 * H], F32, kind="ExternalInput")
    wp_d = nc.dram_tensor("W_proj", [H, H], BF16, kind="ExternalInput")
    bp2_d = nc.dram_tensor("b_proj2", [H], F32, kind="ExternalInput")
    y_d = nc.dram_tensor("y", [T, H], F32, kind="ExternalOutput")

    ident_fn = mybir.ActivationFunctionType.Identity

    with tile.TileContext(nc) as tc:
        with (
            tc.tile_pool(name="sb", bufs=1) as sb,
            tc.tile_pool(name="ps", bufs=1, space="PSUM") as ps,
        ):
            # ---------------- persistent SBUF tensors ----------------
            # One tile per DMA so descriptor-issue cost stays low (issuing a
            # DMA occupies its queue ~0.6us) while keeping producer groups
            # separate so pipelined consumers don't false-share.
            watq01 = sb.tile([128, FT, 256], BF16, tag="watq01")    # q cols nt 0,1
            watk01 = sb.tile([128, FT, 256], BF16, tag="watk01")    # k cols nt 6,7
            watq25 = sb.tile([128, FT, 512], BF16, tag="watq25")    # q cols nt 2-5
            watk25 = sb.tile([128, FT, 512], BF16, tag="watk25")    # k cols nt 8-11
            watv = sb.tile([128, FT, H], BF16, tag="watv")          # v cols
            wpr = sb.tile([128, FT, H], BF16, tag="wpr")
            xTt = [sb.tile([128, 3, T], BF16, tag=f"xT{a}", name=f"xT{a}")
                   for a in range(2)]

            def xT(ft):
                return xTt[ft // 3][:, ft % 3, :]
            kT = [sb.tile([128, T], BF16, tag=f"kT{hp}", name=f"kT{hp}")
                  for hp in range(NH // 2)]
            # q^T zero-padded per head: head h occupies rows 64*(h%2)..+64
            qTp = [sb.tile([128, T], BF16, tag=f"qTp{h}", name=f"qTp{h}")
                   for h in range(NH)]
            # v natural [token, head*(64+1)+...]: [v_h | 1] blocks + 64 pad
            v_t = [sb.tile([128, NH * (HS + 1) + 64], BF16, tag=f"v{tt}", name=f"v{tt}")
                   for tt in range(TT)]
            oT = sb.tile([128, FT, T], BF16, tag="oT")              # attn out^T
            bcols = sb.tile([128, 12], F32, tag="bcols")            # b_qk as columns
            bp_row = sb.tile([1, H], F32, tag="bp_row")
            bp_rowb = sb.tile([1, H], BF16, tag="bp_rowb")
            ones0 = sb.tile([1, 128], BF16, tag="ones0")            # ones at partition 0
            ones64 = sb.tile([65, 128], BF16, tag="ones64")         # row 64 = ones
            tri = sb.tile([128, 128], BF16, tag="tri")              # upper-tri (p<=f) of 1.0
            tri2 = sb.tile([128, 2, 128], BF16, tag="tri2")         # tri duplicated

            def wat_qk(ft, nt):
                """AP holding W_attn cols [128*nt, 128*(nt+1)) for k-tile ft."""
                if nt < 2:
                    return watq01[:, ft, 128 * nt:128 * nt + 128]
                if nt < 6:
                    return watq25[:, ft, 128 * (nt - 2):128 * (nt - 2) + 128]
                if nt < 8:
                    return watk01[:, ft, 128 * (nt - 6):128 * (nt - 6) + 128]
                return watk25[:, ft, 128 * (nt - 8):128 * (nt - 8) + 128]

            # ---------------- constants ----------------
            # tri + the big qTp zero-pads on gpsimd (done before the first
            # causal mask needs the queue); the tiny v/ones memsets on DVE
            make_upper_triangular(nc, tri[:], val=1.0, diag=True)
            for a in range(2):
                nc.gpsimd.tensor_copy(tri2[:, a, :], tri[:])
            nc.vector.memset(ones0[:], 1.0)
            nc.vector.memset(ones64[64:65, :], 1.0)
            # q-tile zero-pads are emitted lazily inside emit_Q so gpsimd
            # interleaves them with the causal masks by actual need order
            qTp_zeroed = set()
            for tt in range(TT):
                nc.vector.memset(v_t[tt][:, NH * (HS + 1):], 0.0)      # tail pad
                nc.vector.memset(v_t[tt][:, HS:NH * (HS + 1):HS + 1], 1.0)  # ones cols

            # ---------------- input DMAs ----------------
            # One DMA per tensor group (descriptor issue is ~0.6us of queue
            # time each), split across both HWDGE queues, ordered by first
            # use: x^T and the q01/k01 weight columns feed head pair 0.
            def pft(ap):  # DRAM rows (f p) -> partition-major 3D
                return ap.rearrange("(f p) c -> p f c", p=128)

            # x^T per-ft (cheap 2D descriptors) split across both queues in
            # parallel; bcols + q01/k01 weight groups lead their queues so
            # head pair 0's chains and evacs have operands ASAP. The scalar
            # queue is kept short so the ACT engine is free for early evacs
            # and the first exps.
            nc.scalar.dma_start(watq01[:], pft(wa_d[:, 0:256]))
            for ft in range(3):
                nc.sync.dma_start(xTt[0][:, ft, :], xT_d[ft * 128:(ft + 1) * 128, :])
                nc.scalar.dma_start(xTt[1][:, ft, :], xT_d[(ft + 3) * 128:(ft + 4) * 128, :])
            nc.sync.dma_start(watk01[:], pft(wa_d[:, 768:1024]))
            nc.sync.dma_start(watv[:], pft(wa_d[:, 2 * H:]))
            nc.sync.dma_start(watk25[:], pft(wa_d[:, 1024:1536]))
            nc.scalar.dma_start(bcols[:], bqk_d[: 12 * 128].rearrange("(t p) -> p t", p=128))
            nc.scalar.dma_start(watq25[:], pft(wa_d[:, 256:768]))
            nc.scalar.dma_start(wpr[:], pft(wp_d[:, :]))
            nc.scalar.dma_start(bp_row[:], bp2_d[None, :])
            nc.vector.tensor_copy(bp_rowb[:], bp_row[:])

            # ---------------- pipelined QKV-projection units ----------------
            ident_fn = mybir.ActivationFunctionType.Identity

            def emit_Q(nt, tg, act_evac=False):
                """q/k projection chain for feature block nt, token group tg."""
                if nt < 6 and nt not in qTp_zeroed:
                    qTp_zeroed.add(nt)
                    nc.gpsimd.memset(qTp[2 * nt][:], 0.0)
                    nc.gpsimd.memset(qTp[2 * nt + 1][:], 0.0)
                pq = ps.tile([128, 512], F32, tag="op", bufs=4, name="pq")
                sl = slice(tg * 512, (tg + 1) * 512)
                for ft in range(FT):
                    nc.tensor.matmul(
                        pq[:], wat_qk(ft, nt), xT(ft)[:, sl],
                        start=(ft == 0), stop=(ft == FT - 1),
                    )
                # evacs on DVE so the ACT engine stays exp-only during attn;
                # the startup chains use ACT (idle then) to shorten the
                # critical path into the first scores
                if nt < 6:  # q: split halves into per-head zero-padded tiles
                    if act_evac:
                        nc.scalar.activation(
                            qTp[2 * nt][:64, sl], pq[:64, :], ident_fn,
                            bias=bcols[:64, nt:nt + 1])
                        nc.scalar.activation(
                            qTp[2 * nt + 1][64:, sl], pq[64:, :], ident_fn,
                            bias=bcols[64:, nt:nt + 1])
                    else:
                        nc.vector.tensor_scalar_add(
                            qTp[2 * nt][:64, sl], pq[:64, :], bcols[:64, nt:nt + 1])
                        nc.vector.tensor_scalar_add(
                            qTp[2 * nt + 1][64:, sl], pq[64:, :], bcols[64:, nt:nt + 1])
                else:
                    # k: keep head-pair tiles. Evac on ACT so it lands in
                    # parallel with the q evacs on DVE — the k tile is on the
                    # critical path into the next head pair's first scores,
                    # and DVE is near-saturated during attention.
                    nc.scalar.activation(
                        kT[nt - 6][:, sl], pq[:], ident_fn,
                        bias=bcols[:, nt:nt + 1])

            def emit_V(tt, ng):
                """v projection for token tile tt, head group ng (wide streams)."""
                w = 512 if ng == 0 else 256
                pv = ps.tile([128, 512], F32, tag="op", bufs=4, name="pv")
                for ft in range(FT):
                    nc.tensor.matmul(
                        pv[:, :w],
                        xT(ft)[:, tt * 128:(tt + 1) * 128],
                        watv[:, ft, ng * 512:ng * 512 + w],
                        start=(ft == 0), stop=(ft == FT - 1),
                    )
                v3 = v_t[tt][:, :NH * (HS + 1)].rearrange("p (h c) -> p h c", c=HS + 1)
                hlo, hhi = (0, 8) if ng == 0 else (8, 12)
                nc.vector.tensor_copy(
                    v3[:, hlo:hhi, :HS],
                    pv[:, :w].rearrange("p (h d) -> p h d", d=HS),
                )

            # Two filler queues. V units do their PSUM evac on DVE, so they
            # are safe to emit inside the scores/exp ping-pong (ACT stays
            # exp-only there); Q units carry ACT evacs and are emitted at
            # AV-block boundaries where ACT is otherwise idle.
            fillV = deque()  # (cycles, tt, closure)
            fillQ = deque()  # (cycles, nt, closure)
            done = set()

            def _pop(src):
                c, i, fn = src.popleft()
                fn()
                kind = "V" if src is fillV else "Q"
                if not src or src[0][1] != i:
                    done.add((kind, i))
                return c

            def fill(cycles, pref):
                while cycles > 0 and (fillV or fillQ):
                    if pref == "V":
                        src = fillV if fillV else fillQ
                    else:
                        src = fillQ if fillQ else fillV
                    cycles -= _pop(src)

            def drain_V(tt):
                while fillV and ("V", tt) not in done:
                    _pop(fillV)

            def drain_Q(nt):
                while fillQ and ("Q", nt) not in done:
                    _pop(fillQ)

            # upfront: q/k operands of head pair 0, and the v tiles its AV
            # needs first (their DVE evacs queue ahead of the attn traffic)
            for nt in (0, 6):
                for tg in range(2):
                    emit_Q(nt, tg)
            for tt in range(4):
                emit_V(tt, 0)
            done.update({("Q", 0), ("Q", 6)})
            done.update({("V", (tt, 0)) for tt in range(4)})

            # ng-major: heads 0-7 (ng0) feed A(hp<4); heads 8-11 by A(4,0)
            for tt in range(4, TT):
                fillV.append((3072, (tt, 0), lambda tt=tt: emit_V(tt, 0)))
            for tt in range(TT):
                fillV.append((1536, (tt, 1), lambda tt=tt: emit_V(tt, 1)))
            for nt in (1, 7, 2, 8, 3, 9, 4, 10, 5, 11):
                for tg in range(2):
                    fillQ.append((3072, nt, lambda nt=nt, tg=tg: emit_Q(nt, tg)))

            # ---------------- attention (per head pair) ----------------
            # Deferred normalize: the recip -> broadcast-matmul -> multiply
            # chain of each group is flushed one group later so the PE never
            # head-of-line blocks on the DVE reciprocal.
            def norm_flush(pending):
                for hi, hp_, qg_, op_, recb_ in pending:
                    base = 64 * hi
                    bp = ps.tile([128, 512], F32, tag="op", bufs=4, name="bp")
                    nc.tensor.matmul(
                        bp[:], ones64[64:65, :], recb_[64:65, :],
                        start=True, stop=True,
                    )
                    bpb = sb.tile([64, 512], BF16, tag="bpb", bufs=2, name="bpb")
                    nc.vector.tensor_copy(bpb[:], bp[:64, :])
                    dst = slice(512 * qg_, 512 * (qg_ + 1))
                    if hi == 0:
                        nc.vector.tensor_mul(oT[:64, hp_, dst], op_[:64, :], bpb[:])
                    else:
                        sc = sb.tile([64, 512], BF16, tag="sc", bufs=3, name="sc")
                        nc.vector.tensor_mul(sc[:], op_[:64, :], bpb[:])
                        nc.sync.dma_start(oT[base:base + 64, hp_, dst], sc[:])

            with nc.named_scope("attn"):
                pending = []
                for hp in range(NH // 2):
                    if hp >= 1:
                        drain_Q(hp)
                        drain_Q(6 + hp)
                    if hp == 4:  # AV for heads 8-11 needs the ng1 v columns
                        drain_V((TT - 1, 1))
                    for qg in range(2):
                        kts = list(range(4 * qg + 4))
                        # P^T for both heads of the pair: [p, hi, kt-slot, q]
                        pts = sb.tile([128, 2, 8, 512], BF16, tag="pT", bufs=2, name="pts")
                        for kp in range(0, len(kts), 2):
                            kt0, kt1 = kts[kp], kts[kp + 1]
                            offs, ws = [], []
                            for j, kt in enumerate((kt0, kt1)):
                                q_off = max(128 * kt, 512 * qg)
                                offs.append(q_off)
                                ws.append(512 * (qg + 1) - q_off)
                            vw = 512 + ws[1]  # exp span: slot0 prefix + slot1 valid part
                            sps2 = [
                                ps.tile([128, 1024], F32, tag="sp0", bufs=1, name="spA"),
                                ps.tile([128, 1024], F32, tag="sp1", bufs=1, name="spB"),
                            ]
                            for j, kt in enumerate((kt0, kt1)):
                                for hi in range(2):  # same lhsT back-to-back
                                    nc.tensor.matmul(
                                        sps2[hi][:, j * 512:j * 512 + ws[j]],
                                        kT[hp][:, kt * 128:(kt + 1) * 128],
                                        qTp[2 * hp + hi][:, offs[j]:offs[j] + ws[j]],
                                        start=True,
                                        stop=True,
                                    )
                            for hi in range(2):
                                dst = pts[:, hi, kt0:kt0 + 2, :].rearrange("p a b -> p (a b)")
                                nc.scalar.activation(
                                    dst[:, :vw], sps2[hi][:, :vw],
                                    mybir.ActivationFunctionType.Exp, scale=0.125,
                                )
                                if 128 * kt0 >= 512 * qg:  # diagonal tiles: causal mask
                                    nc.gpsimd.tensor_mul(
                                        pts[:, hi, kt0:kt0 + 2, :128],
                                        pts[:, hi, kt0:kt0 + 2, :128], tri2[:])
                            # keep the PE fed while ACT drains this PSUM pair.
                            # hp0 needs its v tiles first; afterwards prefer
                            # q/k chains so the NEXT head pair's DVE evacs
                            # finish well before its scores need them (the
                            # remaining v columns aren't needed until hp4)
                            if kp + 2 < len(kts):
                                fill(2600, "V" if hp == 0 else "Q")
                        if hp == 0:  # AV below reads v tiles 0..4*qg+3
                            for tt in range(4 * qg + 4):
                                drain_V((tt, 0))
                        norm_flush(pending)
                        pending = []
                        # the two heads' AV chains interleave so the second
                        # chain's pipeline fill hides under the first's stream
                        ops = [ps.tile([128, 512], F32, tag="op", bufs=4, name=f"av{hi}")
                               for hi in range(2)]
                        for j, kt in enumerate(kts):
                            q_off = max(128 * kt, 512 * qg)
                            w = 512 * (qg + 1) - q_off
                            off = q_off - 512 * qg
                            for hi in range(2):
                                h = 2 * hp + hi
                                nc.tensor.matmul(
                                    ops[hi][:, off:off + w],
                                    v_t[kt][:, 65 * h:65 * h + 128],
                                    pts[:, hi, kt, :w],
                                    start=(j == 0),
                                    stop=(j == len(kts) - 1),
                                )
                        for hi in range(2):
                            # reciprocal of denominator (row 64), stays on partition 64
                            rec = sb.tile([65, 512], F32, tag="rec", bufs=2)
                            recb = sb.tile([65, 512], BF16, tag="recb", bufs=2)
                            nc.vector.reciprocal_approx_fast(rec[:, :], ops[hi][:65, :])
                            nc.vector.tensor_copy(recb[64:65, :], rec[64:65, :])
                            pending.append((hi, hp, qg, ops[hi], recb))
                # any leftover projection work (normally none)
                while fillV or fillQ:
                    fill(1 << 30, "Q")

            # ---------------- output projection ----------------
            def emit_proj(tt):
                ysb = sb.tile([128, H], F32, tag="ysb", bufs=2, name="ysb")
                pys = [
                    ps.tile([128, 512], F32, tag="op", bufs=4, name="py0"),
                    ps.tile([128, 512], F32, tag="op", bufs=4, name="py1"),
                ]
                for ft in range(FT):
                    for ng in range(2):
                        w = 512 if ng == 0 else 256
                        nc.tensor.matmul(
                            pys[ng][:, :w],
                            oT[:, ft, tt * 128:(tt + 1) * 128],
                            wpr[:, ft, ng * 512:ng * 512 + w],
                            start=(ft == 0),
                            stop=False,
                        )
                for ng in range(2):
                    w = 512 if ng == 0 else 256
                    nc.tensor.matmul(
                        pys[ng][:, :w],
                        ones0[:1, :],
                        bp_rowb[:1, ng * 512:ng * 512 + w],
                        start=False,
                        stop=True,
                    )
                    nc.scalar.copy(ysb[:, ng * 512:ng * 512 + w], pys[ng][:, :w])
                nc.sync.dma_start(y_d[tt * 128:(tt + 1) * 128, :], ysb[:])

            with nc.named_scope("proj"):
                # token tiles 0-1 only need the q-group-0 slices of oT, which
                # the loop above already flushed; emit them under the final
                # (deferred) normalize of the last head pair
                emit_proj(0)
                emit_proj(1)
                norm_flush(pending)
                for tt in range(2, TT):
                    emit_proj(tt)

    nc.compile()
    return nc


_NC = None


def _run(in_maps, trace=False, **kwargs):
    global _NC
    if _NC is None:
        _NC = build()
    return bass_utils.run_bass_kernel_spmd(
        _NC, in_maps, core_ids=list(range(N_CORES)), trace=trace, **kwargs
    )


def make_in_maps(x, W_attn, b_attn, W_proj, b_proj):
    import ml_dtypes
    bf = ml_dtypes.bfloat16
    x = np.asarray(x, dtype=np.float32).astype(bf)
    W_attn_b = np.ascontiguousarray(np.asarray(W_attn, dtype=np.float32).astype(bf))
    b_attn = np.asarray(b_attn, dtype=np.float32)
    W_proj_f = np.asarray(W_proj, dtype=np.float32)
    W_proj_b = np.ascontiguousarray(W_proj_f.astype(bf))
    b_qk = np.ascontiguousarray(b_attn[:2 * H])
    # softmax rows sum to 1, so the v bias passes through attention unchanged
    # and can be folded into the projection bias exactly (fp32 on host).
    b_proj2 = np.ascontiguousarray(
        b_attn[2 * H:] @ W_proj_f + np.asarray(b_proj, dtype=np.float32))
    return [
        {
            "xT": np.ascontiguousarray(x[b].T),
            "W_attn": W_attn_b,
            "b_qk": b_qk,
            "W_proj": W_proj_b,
            "b_proj2": b_proj2,
        }
        for b in range(N_CORES)
    ]


def kernel(x, W_attn, b_attn, W_proj, b_proj):
    in_maps = make_in_maps(x, W_attn, b_attn, W_proj, b_proj)
    res = _run(in_maps, trace=False)
    return np.stack([res.results[b]["y"] for b in range(N_CORES)]).astype(np.float32)


# revision 52
# speedup vs baseline: 1.1565x; 1.1565x over previous
"""Multi-head causal self-attention (GPT-style block) on 8 Trainium2 NeuronCores.

Strategy: data-parallel over batch (B=8 -> 1 batch element per core), weights
replicated. Host-side prep: x is transposed to x^T (so no PE transposes are
needed on device) and the value-projection bias is folded into the output
bias (b_proj' = b_v @ W_proj + b_proj, exact in fp32), since softmax rows
sum to 1.

Per-core dataflow (all matmul compute bf16 with fp32 PSUM accumulation):

  qT/kT [n,T] = W_attn[:, n].T-stationary matmuls over xT   (n on partitions)
  v    [T,n] = xT-stationary matmuls over W_attn v-cols     (T on partitions)
  scores^T [k,q] = kT_h.T @ qT_h  (K=64 contraction via zero-padded qT)
  P^T = exp(0.125*scores^T) via ACT, causal diag tiles masked by gpsimd mult
  out^T[d,q] & softmax denom = [v_h | ones].T @ P^T  (ones col -> denom row)
  normalize via PE-broadcast of 1/denom, DVE multiply
  y [T,H] = out^T-stationary matmuls over W_proj + bias

The QKV-projection work (q/k and v matmul chains) is software-pipelined into
the attention phase: emitted as "filler" between score-matmul groups so the
PE stays busy while ACT computes exp (PSUM ping-pong would otherwise stall
it). PSUM evacuations are spread across DVE (qk with per-partition bias add,
v copies) and ACT (exp, proj) to keep every engine below the PE's busy time.
"""

from collections import deque

import numpy as np

import concourse.bass as bass
import concourse.mybir as mybir
import concourse.tile as tile
from concourse import bacc, bass_utils
from concourse.masks import make_upper_triangular

F32 = mybir.dt.float32
BF16 = mybir.dt.bfloat16

T = 1024   # tokens per batch element
H = 768    # hidden
NH = 12    # heads
HS = 64    # head size
TT = T // 128   # token tiles (8)
FT = H // 128   # feature tiles (6)
N_CORES = 8


def build():
    nc = bacc.Bacc(None, target_bir_lowering=False)

    xT_d = nc.dram_tensor("xT", [H, T], BF16, kind="ExternalInput")
    wa_d = nc.dram_tensor("W_attn", [H, 3 * H], BF16, kind="ExternalInput")
    bqk_d = nc.dram_tensor("b_qk", [2 * H], F32, kind="ExternalInput")
    wp_d = nc.dram_tensor("W_proj", [H, H], BF16, kind="ExternalInput")
    bp2_d = nc.dram_tensor("b_proj2", [H], F32, kind="ExternalInput")
    y_d = nc.dram_tensor("y", [T, H], F32, kind="ExternalOutput")

    ident_fn = mybir.ActivationFunctionType.Identity

    with tile.TileContext(nc) as tc:
        with (
            tc.tile_pool(name="sb", bufs=1) as sb,
            tc.tile_pool(name="ps", bufs=1, space="PSUM") as ps,
        ):
            # ---------------- persistent SBUF tensors ----------------
            # One tile per DMA so descriptor-issue cost stays low (issuing a
            # DMA occupies its queue ~0.6us) while keeping producer groups
            # separate so pipelined consumers don't false-share.
            watq01 = sb.tile([128, FT, 256], BF16, tag="watq01")    # q cols nt 0,1
            watk01 = sb.tile([128, FT, 256], BF16, tag="watk01")    # k cols nt 6,7
            watq25 = sb.tile([128, FT, 512], BF16, tag="watq25")    # q cols nt 2-5
            watk25 = sb.tile([128, FT, 512], BF16, tag="watk25")    # k cols nt 8-11
            watv = sb.tile([128, FT, H], BF16, tag="watv")          # v cols
            wpr = sb.tile([128, FT, H], BF16, tag="wpr")
            xTt = [sb.tile([128, 3, T], BF16, tag=f"xT{a}", name=f"xT{a}")
                   for a in range(2)]

            def xT(ft):
                return xTt[ft // 3][:, ft % 3, :]
            kT = [sb.tile([128, T], BF16, tag=f"kT{hp}", name=f"kT{hp}")
                  for hp in range(NH // 2)]
            # q^T zero-padded per head: head h occupies rows 64*(h%2)..+64
            qTp = [sb.tile([128, T], BF16, tag=f"qTp{h}", name=f"qTp{h}")
                   for h in range(NH)]
            # v natural [token, head*(64+1)+...]: [v_h | 1] blocks + 64 pad
            v_t = [sb.tile([128, NH * (HS + 1) + 64], BF16, tag=f"v{tt}", name=f"v{tt}")
                   for tt in range(TT)]
            oT = sb.tile([128, FT, T], BF16, tag="oT")              # attn out^T
            bcols = sb.tile([128, 12], F32, tag="bcols")            # b_qk as columns
            bp_row = sb.tile([1, H], F32, tag="bp_row")
            bp_rowb = sb.tile([1, H], BF16, tag="bp_rowb")
            ones0 = sb.tile([1, 128], BF16, tag="ones0")            # ones at partition 0
            ones64 = sb.tile([65, 128], BF16, tag="ones64")         # row 64 = ones
            tri = sb.tile([128, 128], BF16, tag="tri")              # upper-tri (p<=f) of 1.0
            tri2 = sb.tile([128, 2, 128], BF16, tag="tri2")         # tri duplicated

            def wat_qk(ft, nt):
                """AP holding W_attn cols [128*nt, 128*(nt+1)) for k-tile ft."""
                if nt < 2:
                    return watq01[:, ft, 128 * nt:128 * nt + 128]
                if nt < 6:
                    return watq25[:, ft, 128 * (nt - 2):128 * (nt - 2) + 128]
                if nt < 8:
                    return watk01[:, ft, 128 * (nt - 6):128 * (nt - 6) + 128]
                return watk25[:, ft, 128 * (nt - 8):128 * (nt - 8) + 128]

            # ---------------- constants ----------------
            # tri + the big qTp zero-pads on gpsimd (done before the first
            # causal mask needs the queue); the tiny v/ones memsets on DVE
            make_upper_triangular(nc, tri[:], val=1.0, diag=True)
            for a in range(2):
                nc.gpsimd.tensor_copy(tri2[:, a, :], tri[:])
            nc.vector.memset(ones0[:], 1.0)
            nc.vector.memset(ones64[64:65, :], 1.0)
            # q-tile zero-pads are emitted lazily inside emit_Q so gpsimd
            # interleaves them with the causal masks by actual need order
            qTp_zeroed = set()
            for tt in range(TT):
                nc.vector.memset(v_t[tt][:, NH * (HS + 1):], 0.0)      # tail pad
                nc.vector.memset(v_t[tt][:, HS:NH * (HS + 1):HS + 1], 1.0)  # ones cols

            # ---------------- input DMAs ----------------
            # One DMA per tensor group (descriptor issue is ~0.6us of queue
            # time each), split across both HWDGE queues, ordered by first
            # use: x^T and the q01/k01 weight columns feed head pair 0.
            def pft(ap):  # DRAM rows (f p) -> partition-major 3D
                return ap.rearrange("(f p) c -> p f c", p=128)

            # x^T per-ft (cheap 2D descriptors) split across both queues in
            # parallel; bcols + q01/k01 weight groups lead their queues so
            # head pair 0's chains and evacs have operands ASAP. The scalar
            # queue is kept short so the ACT engine is free for early evacs
            # and the first exps.
            nc.scalar.dma_start(watq01[:], pft(wa_d[:, 0:256]))
            for ft in range(3):
                nc.sync.dma_start(xTt[0][:, ft, :], xT_d[ft * 128:(ft + 1) * 128, :])
                nc.scalar.dma_start(xTt[1][:, ft, :], xT_d[(ft + 3) * 128:(ft + 4) * 128, :])
            nc.sync.dma_start(watk01[:], pft(wa_d[:, 768:1024]))
            nc.sync.dma_start(watv[:], pft(wa_d[:, 2 * H:]))
            nc.sync.dma_start(watk25[:], pft(wa_d[:, 1024:1536]))
            nc.scalar.dma_start(bcols[:], bqk_d[: 12 * 128].rearrange("(t p) -> p t", p=128))
            nc.scalar.dma_start(watq25[:], pft(wa_d[:, 256:768]))
            nc.scalar.dma_start(wpr[:], pft(wp_d[:, :]))
            nc.scalar.dma_start(bp_row[:], bp2_d[None, :])
            nc.vector.tensor_copy(bp_rowb[:], bp_row[:])

            # ---------------- pipelined QKV-projection units ----------------
            ident_fn = mybir.ActivationFunctionType.Identity

            def emit_Q(nt, tg, act_evac=False):
                """q/k projection chain for feature block nt, token group tg."""
                if nt < 6 and nt not in qTp_zeroed:
                    qTp_zeroed.add(nt)
                    nc.gpsimd.memset(qTp[2 * nt][:], 0.0)
                    nc.gpsimd.memset(qTp[2 * nt + 1][:], 0.0)
                pq = ps.tile([128, 512], F32, tag="op", bufs=4, name="pq")
                sl = slice(tg * 512, (tg + 1) * 512)
                for ft in range(FT):
                    nc.tensor.matmul(
                        pq[:], wat_qk(ft, nt), xT(ft)[:, sl],
                        start=(ft == 0), stop=(ft == FT - 1),
                    )
                # evacs on DVE so the ACT engine stays exp-only during attn;
                # the startup chains use ACT (idle then) to shorten the
                # critical path into the first scores
                if nt < 6:  # q: split halves into per-head zero-padded tiles
                    if act_evac:
                        nc.scalar.activation(
                            qTp[2 * nt][:64, sl], pq[:64, :], ident_fn,
                            bias=bcols[:64, nt:nt + 1])
                        nc.scalar.activation(
                            qTp[2 * nt + 1][64:, sl], pq[64:, :], ident_fn,
                            bias=bcols[64:, nt:nt + 1])
                    else:
                        nc.vector.tensor_scalar_add(
                            qTp[2 * nt][:64, sl], pq[:64, :], bcols[:64, nt:nt + 1])
                        nc.vector.tensor_scalar_add(
                            qTp[2 * nt + 1][64:, sl], pq[64:, :], bcols[64:, nt:nt + 1])
                else:       # k: keep head-pair tiles
                    nc.vector.tensor_scalar_add(
                        kT[nt - 6][:, sl], pq[:], bcols[:, nt:nt + 1])

            def emit_V(tt, ng):
                """v projection for token tile tt, head group ng (wide streams)."""
                w = 512 if ng == 0 else 256
                pv = ps.tile([128, 512], F32, tag="op", bufs=4, name="pv")
                for ft in range(FT):
                    nc.tensor.matmul(
                        pv[:, :w],
                        xT(ft)[:, tt * 128:(tt + 1) * 128],
                        watv[:, ft, ng * 512:ng * 512 + w],
                        start=(ft == 0), stop=(ft == FT - 1),
                    )
                v3 = v_t[tt][:, :NH * (HS + 1)].rearrange("p (h c) -> p h c", c=HS + 1)
                hlo, hhi = (0, 8) if ng == 0 else (8, 12)
                nc.vector.tensor_copy(
                    v3[:, hlo:hhi, :HS],
                    pv[:, :w].rearrange("p (h d) -> p h d", d=HS),
                )

            # Two filler queues. V units do their PSUM evac on DVE, so they
            # are safe to emit inside the scores/exp ping-pong (ACT stays
            # exp-only there); Q units carry ACT evacs and are emitted at
            # AV-block boundaries where ACT is otherwise idle.
            fillV = deque()  # (cycles, tt, closure)
            fillQ = deque()  # (cycles, nt, closure)
            done = set()

            def _pop(src):
                c, i, fn = src.popleft()
                fn()
                kind = "V" if src is fillV else "Q"
                if not src or src[0][1] != i:
                    done.add((kind, i))
                return c

            def fill(cycles, pref):
                while cycles > 0 and (fillV or fillQ):
                    if pref == "V":
                        src = fillV if fillV else fillQ
                    else:
                        src = fillQ if fillQ else fillV
                    cycles -= _pop(src)

            def drain_V(tt):
                while fillV and ("V", tt) not in done:
                    _pop(fillV)

            def drain_Q(nt):
                while fillQ and ("Q", nt) not in done:
                    _pop(fillQ)

            # upfront: q/k operands of head pair 0, and the v tiles its AV
            # needs first (their DVE evacs queue ahead of the attn traffic)
            for nt in (0, 6):
                for tg in range(2):
                    emit_Q(nt, tg)
            for tt in range(4):
                emit_V(tt, 0)
            done.update({("Q", 0), ("Q", 6)})
            done.update({("V", (tt, 0)) for tt in range(4)})

            # ng-major: heads 0-7 (ng0) feed A(hp<4); heads 8-11 by A(4,0)
            for tt in range(4, TT):
                fillV.append((3072, (tt, 0), lambda tt=tt: emit_V(tt, 0)))
            for tt in range(TT):
                fillV.append((1536, (tt, 1), lambda tt=tt: emit_V(tt, 1)))
            for nt in (1, 7, 2, 8, 3, 9, 4, 10, 5, 11):
                for tg in range(2):
                    fillQ.append((3072, nt, lambda nt=nt, tg=tg: emit_Q(nt, tg)))

            # ---------------- attention (per head pair) ----------------
            # Deferred normalize: the recip -> broadcast-matmul -> multiply
            # chain of each group is flushed one group later so the PE never
            # head-of-line blocks on the DVE reciprocal.
            def norm_flush(pending):
                for hi, hp_, qg_, op_, recb_ in pending:
                    base = 64 * hi
                    bp = ps.tile([128, 512], F32, tag="op", bufs=4, name="bp")
                    nc.tensor.matmul(
                        bp[:], ones64[64:65, :], recb_[64:65, :],
                        start=True, stop=True,
                    )
                    bpb = sb.tile([64, 512], BF16, tag="bpb", bufs=2, name="bpb")
                    nc.vector.tensor_copy(bpb[:], bp[:64, :])
                    dst = slice(512 * qg_, 512 * (qg_ + 1))
                    if hi == 0:
                        nc.vector.tensor_mul(oT[:64, hp_, dst], op_[:64, :], bpb[:])
                    else:
                        sc = sb.tile([64, 512], BF16, tag="sc", bufs=3, name="sc")
                        nc.vector.tensor_mul(sc[:], op_[:64, :], bpb[:])
                        nc.sync.dma_start(oT[base:base + 64, hp_, dst], sc[:])

            with nc.named_scope("attn"):
                pending = []
                for hp in range(NH // 2):
                    if hp >= 1:
                        drain_Q(hp)
                        drain_Q(6 + hp)
                    if hp == 4:  # AV for heads 8-11 needs the ng1 v columns
                        drain_V((TT - 1, 1))
                    for qg in range(2):
                        kts = list(range(4 * qg + 4))
                        # P^T for both heads of the pair: [p, hi, kt-slot, q]
                        pts = sb.tile([128, 2, 8, 512], BF16, tag="pT", bufs=2, name="pts")
                        for kp in range(0, len(kts), 2):
                            kt0, kt1 = kts[kp], kts[kp + 1]
                            offs, ws = [], []
                            for j, kt in enumerate((kt0, kt1)):
                                q_off = max(128 * kt, 512 * qg)
                                offs.append(q_off)
                                ws.append(512 * (qg + 1) - q_off)
                            vw = 512 + ws[1]  # exp span: slot0 prefix + slot1 valid part
                            sps2 = [
                                ps.tile([128, 1024], F32, tag="sp0", bufs=1, name="spA"),
                                ps.tile([128, 1024], F32, tag="sp1", bufs=1, name="spB"),
                            ]
                            for j, kt in enumerate((kt0, kt1)):
                                for hi in range(2):  # same lhsT back-to-back
                                    nc.tensor.matmul(
                                        sps2[hi][:, j * 512:j * 512 + ws[j]],
                                        kT[hp][:, kt * 128:(kt + 1) * 128],
                                        qTp[2 * hp + hi][:, offs[j]:offs[j] + ws[j]],
                                        start=True,
                                        stop=True,
                                    )
                            for hi in range(2):
                                dst = pts[:, hi, kt0:kt0 + 2, :].rearrange("p a b -> p (a b)")
                                nc.scalar.activation(
                                    dst[:, :vw], sps2[hi][:, :vw],
                                    mybir.ActivationFunctionType.Exp, scale=0.125,
                                )
                                if 128 * kt0 >= 512 * qg:  # diagonal tiles: causal mask
                                    nc.gpsimd.tensor_mul(
                                        pts[:, hi, kt0:kt0 + 2, :128],
                                        pts[:, hi, kt0:kt0 + 2, :128], tri2[:])
                            # keep the PE fed while ACT drains this PSUM pair
                            if kp + 2 < len(kts):
                                fill(2600, "V")
                        if hp == 0:  # AV below reads v tiles 0..4*qg+3
                            for tt in range(4 * qg + 4):
                                drain_V((tt, 0))
                        norm_flush(pending)
                        pending = []
                        for hi in range(2):
                            h = 2 * hp + hi
                            op = ps.tile([128, 512], F32, tag="op", bufs=4)
                            for j, kt in enumerate(kts):
                                q_off = max(128 * kt, 512 * qg)
                                w = 512 * (qg + 1) - q_off
                                off = q_off - 512 * qg
                                nc.tensor.matmul(
                                    op[:, off:off + w],
                                    v_t[kt][:, 65 * h:65 * h + 128],
                                    pts[:, hi, kt, :w],
                                    start=(j == 0),
                                    stop=(j == len(kts) - 1),
                                )
                            # reciprocal of denominator (row 64), stays on partition 64
                            rec = sb.tile([65, 512], F32, tag="rec", bufs=2)
                            recb = sb.tile([65, 512], BF16, tag="recb", bufs=2)
                            nc.vector.reciprocal_approx_fast(rec[:, :], op[:65, :])
                            nc.vector.tensor_copy(recb[64:65, :], rec[64:65, :])
                            pending.append((hi, hp, qg, op, recb))
                # any leftover projection work (normally none)
                while fillV or fillQ:
                    fill(1 << 30, "Q")

            # ---------------- output projection ----------------
            def emit_proj(tt):
                ysb = sb.tile([128, H], F32, tag="ysb", bufs=2, name="ysb")
                pys = [
                    ps.tile([128, 512], F32, tag="op", bufs=4, name="py0"),
                    ps.tile([128, 512], F32, tag="op", bufs=4, name="py1"),
                ]
                for ft in range(FT):
                    for ng in range(2):
                        w = 512 if ng == 0 else 256
                        nc.tensor.matmul(
                            pys[ng][:, :w],
                            oT[:, ft, tt * 128:(tt + 1) * 128],
                            wpr[:, ft, ng * 512:ng * 512 + w],
                            start=(ft == 0),
                            stop=False,
                        )
                for ng in range(2):
                    w = 512 if ng == 0 else 256
                    nc.tensor.matmul(
                        pys[ng][:, :w],
                        ones0[:1, :],
                        bp_rowb[:1, ng * 512:ng * 512 + w],
                        start=False,
                        stop=True,
                    )
                    nc.scalar.copy(ysb[:, ng * 512:ng * 512 + w], pys[ng][:, :w])
                nc.sync.dma_start(y_d[tt * 128:(tt + 1) * 128, :], ysb[:])

            with nc.named_scope("proj"):
                # token tiles 0-1 only need the q-group-0 slices of oT, which
                # the loop above already flushed; emit them under the final
                # (deferred) normalize of the last head pair
                emit_proj(0)
                emit_proj(1)
                norm_flush(pending)
                for tt in range(2, TT):
                    emit_proj(tt)

    nc.compile()
    return nc


_NC = None


def _run(in_maps, trace=False, **kwargs):
    global _NC
    if _NC is None:
        _NC = build()
    return bass_utils.run_bass_kernel_spmd(
        _NC, in_maps, core_ids=list(range(N_CORES)), trace=trace, **kwargs
    )


def make_in_maps(x, W_attn, b_attn, W_proj, b_proj):
    import ml_dtypes
    bf = ml_dtypes.bfloat16
    x = np.asarray(x, dtype=np.float32).astype(bf)
    W_attn_b = np.ascontiguousarray(np.asarray(W_attn, dtype=np.float32).astype(bf))
    b_attn = np.asarray(b_attn, dtype=np.float32)
    W_proj_f = np.asarray(W_proj, dtype=np.float32)
    W_proj_b = np.ascontiguousarray(W_proj_f.astype(bf))
    b_qk = np.ascontiguousarray(b_attn[:2 * H])
    # softmax rows sum to 1, so the v bias passes through attention unchanged
    # and can be folded into the projection bias exactly (fp32 on host).
    b_proj2 = np.ascontiguousarray(
        b_attn[2 * H:] @ W_proj_f + np.asarray(b_proj, dtype=np.float32))
    return [
        {
            "xT": np.ascontiguousarray(x[b].T),
            "W_attn": W_attn_b,
            "b_qk": b_qk,
            "W_proj": W_proj_b,
            "b_proj2": b_proj2,
        }
        for b in range(N_CORES)
    ]


def kernel(x, W_attn, b_attn, W_proj, b_proj):
    in_maps = make_in_maps(x, W_attn, b_attn, W_proj, b_proj)
    res = _run(in_maps, trace=False)
    return np.stack([res.results[b]["y"] for b in range(N_CORES)]).astype(np.float32)
